# revision 5
# baseline (speedup 1.0000x reference)
"""Trainium2 Bass kernel for nn_Attention_31267361915369.

Computation (per batch example, T=4096, D=1024):
    h   = tanh(x @ W1.T + b1)          # [T, D]
    s   = h @ w2.T + b2                # [T]
    e   = exp(s)                       # no max-subtraction: |s| <= sum|w2| ~ 26,
                                       # and num/den is exactly shift-invariant
    num = cumsum(e * x, axis=0)        # [T, D]
    den = cumsum(e)                    # [T]
    ctx = num / den
    out = tanh([ctx, x] @ Wc.T + bc)   # [T, D]

Key reformulation: split Wc = [Wc1 | Wc2] (ctx half, x half). Right-matmul
commutes with the t-cumsum, so
    ctx @ Wc1.T = cumsum(e * (x @ Wc1.T)) / den = cumsum(e * y) / den
and
    out = tanh(cumsum(e*y)/den + x @ Wc2.T + bc).
This removes per-tile PE transposes of ctx, drops the natural-layout x
input entirely (only the host-pretransposed xT is read), and makes all
three D x D GEMMs (h_pre, y, x@Wc2.T) depend only on xT and resident
weights, so the TensorE stream is one long uninterrupted sequence (keeps
the PE p-state ramped). The only cross-tile state is a [1, 1025] carry row.

The score GEMM (h_pre) runs in fp8 e4m3 with DoubleRow perf mode (two
k-tiles per instruction, 0.5 cycles/row): the softmax renormalizes the
~4% fp8 quantization error of e^s, and y / x@Wc2.T stay bf16, so the
end-to-end error stays well inside the 2e-2 gate. W1 is pre-scaled by 16
into the e4m3 normal range and descaled by the tanh's scale=1/16.

Distribution: data-parallel over batch B=8 across the 8 NeuronCores (one
example per core), weights replicated. No collectives.

Per-core dataflow (32 token-tiles of 128):
  - per tile, 2 fp8 h chunks + 4 bf16 GEMM chunk-groups of [128, 512]
    rotate through a 4-bank PSUM pool (order h0 h1 w0 w1 y0 y1 so
    e=exp(s) is ready when the y chunks land).
  - causal prefix sums: carry injected into ey row 0 (tri row 0 is all
    ones, so the [128,128] upper-triangular matmul propagates it to every
    row); carry row extracted from PSUM row 127 via ACT copy of the
    [96:128] window + SBUF->SBUF DMA of row 127.
  - stage-skewed emission: tile i's cumsum/tail (stage_b) sits after tile
    i+1's GEMMs in every engine's stream, giving the carry chain a full
    GEMM-tile of slack before the PE needs it.
  - score reduce and the (z*rden)+xwc2 tail each run as one fused DVE
    scalar_tensor_tensor op (all-f32 operands; wide bf16 tensor-tensor DVE
    ops hang on this hw).
  - weight DMAs are ordered by first use (g8, w2r, then G in column-chunk
    major order) so the opening GEMMs aren't stuck behind the full load.
"""

import sys

if "/opt/trn_rl_repo" not in sys.path:
    sys.path.insert(0, "/opt/trn_rl_repo")

from contextlib import ExitStack

import ml_dtypes
import numpy as np

import concourse.bass as bass
import concourse.tile as tile
from concourse import bacc, mybir
from concourse.bass_utils import run_bass_kernel_spmd

P = 128
D = 1024
T_FULL = 4096
N_CORES = 8
W1_SCALE = 16.0

BF = mybir.dt.bfloat16
F32 = mybir.dt.float32
F8 = mybir.dt.float8e4
AFT = mybir.ActivationFunctionType
ALU = mybir.AluOpType
DR = mybir.MatmulPerfMode.DoubleRow

_BUILD_CACHE: dict = {}


def build(T: int = T_FULL, use_b1: bool = False, use_bc: bool = False,
          repeat: int = 1, fp8_h: bool = True):
    """Build + compile the per-core Bass program for a [T, D] shard."""
    key = (T, use_b1, use_bc, repeat, fp8_h)
    if key in _BUILD_CACHE:
        return _BUILD_CACHE[key]
    if use_b1 and fp8_h:
        raise NotImplementedError("b1 bias with fp8 score path")

    assert T % P == 0
    NT = T // P
    NK = D // P           # 8 k-tiles
    NJ = NK // 2          # 4 k-tile pairs for DoubleRow
    NCH = 4 if fp8_h else 6   # bf16 G chunk count (w0 w1 y0 y1 [+ h0 h1])
    WG = NCH * 512

    nc = bacc.Bacc("TRN2", target_bir_lowering=False, debug=False)

    # host-pretransposed x: xt[i, p, k, t] = x[i*128+t, k*128+p]
    xt_d = nc.declare_dram_parameter("xt", [T * NK, P], BF, isOutput=False)
    xt8_d = (nc.declare_dram_parameter("xt8", [T * NK, P], F8, isOutput=False)
             if fp8_h else None)
    # bf16 weights, chunk-major: g[c, k] = [128, 512] block
    g_d = nc.declare_dram_parameter("g", [NCH * NK * P, 512], BF, isOutput=False)
    # fp8 score weights, pair-major: g8[j, p, i, :] = 16*W1.T[(2j+i)*128+p, :]
    g8_d = (nc.declare_dram_parameter("g8", [NJ * P * 2, D], F8, isOutput=False)
            if fp8_h else None)
    w2r_d = nc.declare_dram_parameter("w2r", [P, D], F32, isOutput=False)
    tri_d = nc.declare_dram_parameter("tri", [P, P], BF, isOutput=False)
    b1_d = nc.declare_dram_parameter("b1r", [1, D], BF, isOutput=False) if use_b1 else None
    bc_d = nc.declare_dram_parameter("bcr", [1, D], BF, isOutput=False) if use_bc else None
    out_d = nc.declare_dram_parameter("out", [T, D], F32, isOutput=True)

    xt_t = xt_d.ap().rearrange("(n p k) q -> n p k q", p=P, k=NK)
    xt8_t = (xt8_d.ap().rearrange("(n p k) q -> n p k q", p=P, k=NK)
             if fp8_h else None)
    out_t = out_d.ap().rearrange("(n p) d -> n p d", p=P)
    g_t = g_d.ap().rearrange("(c k p) q -> c k p q", k=NK, p=P)
    g8_t = g8_d.ap().rearrange("(j p i) q -> j p (i q)", p=P, i=2) if fp8_h else None

    with tile.TileContext(nc) as tc, ExitStack() as ctx:
        consts = ctx.enter_context(tc.tile_pool(name="consts", bufs=1))
        xin = ctx.enter_context(tc.tile_pool(name="xin", bufs=4))
        f32w = ctx.enter_context(tc.tile_pool(name="f32w", bufs=6))
        eyp = ctx.enter_context(tc.tile_pool(name="eyp", bufs=2))
        xw2p = ctx.enter_context(tc.tile_pool(name="xw2p", bufs=2))
        outp = ctx.enter_context(tc.tile_pool(name="outp", bufs=2))
        colp = ctx.enter_context(tc.tile_pool(name="colp", bufs=8))
        ecolp = ctx.enter_context(tc.tile_pool(name="ecolp", bufs=3))
        cstp = ctx.enter_context(tc.tile_pool(name="cstp", bufs=2))
        crowp = ctx.enter_context(tc.tile_pool(name="crowp", bufs=2))
        pg = ctx.enter_context(tc.tile_pool(name="pg", bufs=4, space="PSUM"))
        pcp = ctx.enter_context(tc.tile_pool(name="pcp", bufs=1, space="PSUM"))
        pdp = ctx.enter_context(tc.tile_pool(name="pdp", bufs=1, space="PSUM"))

        # tri first (tiny), then the first two x tiles so the opening GEMMs
        # aren't stuck behind the weight load, then weights in first-use
        # order (g8 for the h chunks, w2r, then G chunk-major).
        tri_sb = consts.tile([P, P], BF, tag="tri")
        nc.sync.dma_start(out=tri_sb[:], in_=tri_d.ap())

        xts = {}

        def load_x(i):
            t = xin.tile([P, NK, P], BF, tag="xt")
            nc.sync.dma_start(out=t[:], in_=xt_t[i])
            if fp8_h:
                t8 = xin.tile([P, NK, P], F8, tag="xt8")
                nc.sync.dma_start(out=t8[:], in_=xt8_t[i])
                xts[i] = (t, t8)
            else:
                xts[i] = (t, None)

        load_x(0)
        load_x(1)

        if fp8_h:
            g8_sb = []
            for j in range(NJ):
                t = consts.tile([P, 2, D], F8, tag=f"g8_{j}")
                nc.sync.dma_start(
                    out=t[:].rearrange("p i q -> p (i q)"), in_=g8_t[j]
                )
                g8_sb.append(t)
        w2r_sb = consts.tile([P, D], F32, tag="w2r")
        nc.sync.dma_start(out=w2r_sb[:], in_=w2r_d.ap())
        g_sb = [consts.tile([P, WG], BF, tag=f"g_{k}") for k in range(NK)]
        for c in range(NCH):
            for k in range(NK):
                nc.sync.dma_start(
                    out=g_sb[k][:, c * 512:(c + 1) * 512], in_=g_t[c, k]
                )
        if use_b1:
            b1_sb = consts.tile([1, D], BF, tag="b1")
            nc.sync.dma_start(out=b1_sb[:], in_=b1_d.ap())
        if use_bc:
            bc_sb = consts.tile([1, D], BF, tag="bc")
            nc.sync.dma_start(out=bc_sb[:], in_=bc_d.ap())

        crows = {}

        def gemm_chunk(xT, c, last_k_bias=None):
            """One [128, 512] output chunk of xT.T @ G, 8 accumulating matmuls."""
            pt = pg.tile([P, 512], F32, tag="pg")
            for k in range(NK):
                nc.tensor.matmul(
                    pt[:],
                    xT[:, k, :],
                    g_sb[k][:, c * 512:(c + 1) * 512],
                    start=(k == 0),
                    stop=(k == NK - 1 and last_k_bias is None),
                )
            if last_k_bias is not None:
                nc.tensor.matmul(
                    pt[:], tri_sb[0:1, :], last_k_bias, start=False, stop=True
                )
            return pt

        def h_chunk_fp8(xT8, c):
            """One [128, 512] chunk of 16*(x@W1.T) via fp8 DoubleRow."""
            pt = pg.tile([P, 512], F32, tag="pg")
            for j in range(NJ):
                nc.tensor.matmul(
                    pt[:],
                    xT8[:, 2 * j:2 * j + 2, :],
                    g8_sb[j][:, :, c * 512:(c + 1) * 512],
                    start=(j == 0),
                    stop=(j == NJ - 1),
                    perf_mode=DR,
                )
            return pt

        def stage_a(i):
            """GEMMs + scores + ey for tile i -> (ey, ecol, xw2)."""
            xT, xT8 = xts[i]

            # -- scores: h = tanh(x@W1.T [+ b1]), s = rowsum(h*w2), e = exp(s)
            h_sb = f32w.tile([P, D], F32, tag="h")
            for c in range(2):
                if fp8_h:
                    pt = h_chunk_fp8(xT8, c)
                    nc.scalar.activation(
                        h_sb[:, c * 512:(c + 1) * 512], pt[:], AFT.Tanh,
                        scale=1.0 / W1_SCALE,
                    )
                else:
                    bias = b1_sb[0:1, c * 512:(c + 1) * 512] if use_b1 else None
                    pt = gemm_chunk(xT, 4 + c, bias)
                    nc.scalar.activation(
                        h_sb[:, c * 512:(c + 1) * 512], pt[:], AFT.Tanh
                    )
            scr = f32w.tile([P, D], F32, tag="scr")
            s_col = colp.tile([P, 1], F32, tag="s")
            nc.vector.scalar_tensor_tensor(
                scr[:], h_sb[:], 1.0, w2r_sb[:], ALU.mult, ALU.mult,
                accum_out=s_col[:],
            )
            e_col = colp.tile([P, 1], F32, tag="e")
            nc.scalar.activation(e_col[:], s_col[:], AFT.Exp)
            ecol_bf = ecolp.tile([P, 1], BF, tag="ecol")
            nc.scalar.copy(ecol_bf[:], e_col[:])

            # -- xwc2 = x@Wc2.T [+ bc] (f32 in SBUF for the all-f32 tail op)
            xw2_sb = xw2p.tile([P, D], F32, tag="xw2")
            for c in range(2):
                bias = bc_sb[0:1, c * 512:(c + 1) * 512] if use_bc else None
                pt = gemm_chunk(xT, c, bias)
                nc.scalar.copy(xw2_sb[:, c * 512:(c + 1) * 512], pt[:])

            # -- ey = e * (x@Wc1.T)
            ey_sb = eyp.tile([P, D], BF, tag="ey")
            for c in range(2):
                pt = gemm_chunk(xT, 2 + c)
                nc.vector.tensor_scalar_mul(
                    ey_sb[:, c * 512:(c + 1) * 512], pt[:], e_col[:]
                )
            return ey_sb, ecol_bf, xw2_sb

        def stage_b(i, ey_sb, ecol_bf, xw2_sb):
            """Carry inject + cumsum + tail for tile i."""
            if i > 0:
                nc.vector.tensor_add(
                    ey_sb[0:1, :], ey_sb[0:1, :], crows[i - 1][0:1, 0:D]
                )
                nc.vector.tensor_add(
                    ecol_bf[0:1, :], ecol_bf[0:1, :], crows[i - 1][0:1, D:D + 1]
                )

            pc0 = pcp.tile([P, 512], F32, tag="pc0")
            pc1 = pcp.tile([P, 512], F32, tag="pc1")
            nc.tensor.matmul(pc0[:], tri_sb[:], ey_sb[:, 0:512], start=True, stop=True)
            nc.tensor.matmul(pc1[:], tri_sb[:], ey_sb[:, 512:1024], start=True, stop=True)
            pd = pdp.tile([P, 1], F32, tag="pd")
            nc.tensor.matmul(pd[:], tri_sb[:], ecol_bf[:], start=True, stop=True)

            # -- extract running totals (row 127) for the next tile's carry.
            # engines can't cross partitions (and must start at a 32-aligned
            # row), so stage the [96:128] window then DMA row 127.
            if i < NT - 1:
                cstage = cstp.tile([P, D + 1], BF, tag="cst")
                nc.scalar.copy(cstage[96:128, 0:512], pc0[96:128, :])
                nc.scalar.copy(cstage[96:128, 512:1024], pc1[96:128, :])
                nc.scalar.copy(cstage[96:128, D:D + 1], pd[96:128, :])
                crow = crowp.tile([1, D + 1], BF, tag="crow")
                nc.sync.dma_start(out=crow[0:1, :], in_=cstage[127:128, :])
                crows[i] = crow

            # -- tail: out = tanh(z*rden + xwc2)
            rden = colp.tile([P, 1], F32, tag="rden")
            nc.vector.reciprocal(rden[:], pd[:])
            u_sb = f32w.tile([P, D], F32, tag="u")
            nc.vector.scalar_tensor_tensor(
                u_sb[:, 0:512], pc0[:], rden[:], xw2_sb[:, 0:512],
                ALU.mult, ALU.add,
            )
            nc.vector.scalar_tensor_tensor(
                u_sb[:, 512:1024], pc1[:], rden[:], xw2_sb[:, 512:1024],
                ALU.mult, ALU.add,
            )
            o_sb = outp.tile([P, D], F32, tag="out")
            nc.scalar.activation(o_sb[:], u_sb[:], AFT.Tanh)
            nc.sync.dma_start(out=out_t[i], in_=o_sb[:])

        def whole_pipeline(first):
            crows.clear()
            if not first:
                load_x(0)
                load_x(1)
            pend = None
            for i in range(NT):
                if i + 2 < NT:
                    load_x(i + 2)
                cur = stage_a(i)
                if pend is not None:
                    stage_b(i - 1, *pend)
                pend = cur
                xts.pop(i)
            stage_b(NT - 1, *pend)

        if repeat == 1:
            whole_pipeline(True)
        else:
            with tc.For_i(0, repeat, 1):
                whole_pipeline(False)

    nc.compile()
    _BUILD_CACHE[key] = nc
    return nc


def _bf16(a):
    return np.ascontiguousarray(np.asarray(a, dtype=np.float32)).astype(
        ml_dtypes.bfloat16
    )


def _fp8(a):
    return np.ascontiguousarray(np.asarray(a, dtype=np.float32)).astype(
        ml_dtypes.float8_e4m3
    )


def make_in_maps(x, W1, b1, w2, b2, Wc, bc, T=T_FULL, fp8_h=True):
    """Host-side prep: shard x over batch, pre-transpose/fuse weights."""
    x = np.asarray(x, dtype=np.float32)
    W1 = np.asarray(W1, dtype=np.float32)
    Wc = np.asarray(Wc, dtype=np.float32)
    w2 = np.asarray(w2, dtype=np.float32).reshape(1, -1)
    b1 = np.asarray(b1, dtype=np.float32)
    bc = np.asarray(bc, dtype=np.float32)
    use_b1 = bool(np.any(b1 != 0.0))
    use_bc = bool(np.any(bc != 0.0))
    # b2 shifts every score equally; exp(b2) cancels in num/den.
    if use_b1:
        fp8_h = False

    # bf16 GEMM weights, chunk-major blocks: [Wc2.T | Wc1.T] (+ W1.T if bf16 h)
    cols = [Wc[:, D:].T, Wc[:, :D].T] + ([] if fp8_h else [W1.T])
    gw = np.concatenate(cols, axis=1)           # [1024, 2048 or 3072]
    NCH = gw.shape[1] // 512
    NK = D // P
    # g[c, k] = gw[k*128:(k+1)*128, c*512:(c+1)*512]
    g = _bf16(
        np.ascontiguousarray(
            gw.reshape(NK, P, NCH, 512).transpose(2, 0, 1, 3)
        ).reshape(NCH * NK * P, 512)
    )
    w2r = np.ascontiguousarray(np.broadcast_to(w2, (P, D)).astype(np.float32))
    tri = _bf16(np.triu(np.ones((P, P), np.float32)))
    if fp8_h:
        w1s = W1.T * W1_SCALE                   # [1024, 1024], k rows
        # g8[j, p, i, :] = w1s[(2j+i)*128+p, :]
        g8 = _fp8(
            np.ascontiguousarray(
                w1s.reshape(NK // 2, 2, P, D).transpose(0, 2, 1, 3)
            ).reshape(NK * P, D)
        )

    NT = T // P
    in_maps = []
    for i in range(N_CORES):
        xb = np.ascontiguousarray(x[i, :T, :])
        # xt[i, p, k, t] = x[i*128+t, k*128+p], 2KB-contiguous per partition
        xtf = np.ascontiguousarray(
            xb.reshape(NT, P, NK, P).transpose(0, 3, 2, 1)
        ).reshape(T * NK, P)
        m = {"xt": _bf16(xtf), "g": g, "w2r": w2r, "tri": tri}
        if fp8_h:
            m["xt8"] = _fp8(xtf)
        if use_b1:
            m["b1r"] = _bf16(b1.reshape(1, D))
        if use_bc:
            m["bcr"] = _bf16(bc.reshape(1, D))
        in_maps.append(m)
    return in_maps, use_b1, use_bc, fp8_h


def kernel(x, W1, b1, w2, b2, Wc, bc):
    in_maps, use_b1, use_bc, fp8_h = make_in_maps(x, W1, b1, w2, b2, Wc, bc)
    nc = build(T_FULL, use_b1, use_bc, fp8_h=fp8_h)
    res = run_bass_kernel_spmd(nc, in_maps, core_ids=list(range(N_CORES)))
    out = np.stack([np.asarray(res.results[i]["out"]) for i in range(N_CORES)], axis=0)
    return out.astype(np.float32)
